# revision 25
# baseline (speedup 1.0000x reference)
"""Trainium2 Bass kernel for nn_Correct_PrototypeManager (segment_reduce).

Reference computation:
    pred_lbl = argmax(preds, axis=1)                      # [B, H, W]
    feats_up = bilinear_resize(feats, H, W)               # [B, C, H, W]
    joint[b,k,h,w] = (masks==k) & (pred_lbl==k)
    counts[b,k] = sum_hw joint ; sums[b,k,c] = sum_hw feats_up * joint
    proto = mean_b( sums / (counts + eps) )               # [K, C]

Algebraic transform: bilinear upsample is linear, feats_up = (Uh (x) Uw) @
feats, so sums[k,c] = <U^T joint_k, feats_c>: downsample the one-hot joint
map (256^2 -> 64^2) with the adjoint of the upsample and contract over 4096
coarse pixels. Counts are preserved exactly (rows of U sum to 1).

v1 design notes (vs the f32 baseline at 124 us):
  - Everything on the wires is fp16: preds/mask/feats uploaded as fp16,
    joint/one-hot/U/A/ds tiles fp16. U weights are multiples of 1/8 and the
    stage outputs stay exactly representable, so stages 1/2 are EXACT; the
    only approximations are the fp16 rounding of preds (rare argmax flips)
    and of feats (0.05% on the contraction inputs).
  - DVE (the bottleneck): argmax via a tree of tensor_tensor max ops (2x
    fp16 mode) instead of a strided tensor_reduce (always 1x); one-hot of
    the mask via 21 tensor_scalar is_equal ops (4x mode).
  - feats uploaded pre-transposed (pixel-major) from host: kills 64 PE
    transposes + 17 us of PSUM->SBUF copies.
  - fp16 matmuls are 4x faster than fp32 (1 cycle/col vs 4).
  - stage2 uses a [U|U] stationary so ds lands on all 128 partitions; the
    odd-row shift for the final contraction is an in-partition free-dim
    shift done by the ACT copy (no SBUF->SBUF DMA).
  - eq/mul chunked by class pairs so stage-1 matmuls chase the DVE through
    the image instead of waiting for the full joint map.

Sharding: data-parallel over batch B=8, one image per NeuronCore; the
[C+1, K] per-image partial (sums^T stacked with counts) is combined on host.
"""

import numpy as np

B = 8
C = 256
K = 21
HC = WC = 64
HF = WF = 256
EPS = 1e-6
N_CORES = 8
PIX = HC * WC  # 4096
KW = K * WF    # 5376
KA = 11        # preds DMA k-chunk split: classes [0,KA) then [KA,K)

_PROGRAM_CACHE: dict = {}


def _upsample_matrix(n_in: int, n_out: int) -> np.ndarray:
    """U [n_out, n_in] with resize(x, 'bilinear', half-pixel) == U @ x."""
    U = np.zeros((n_out, n_in), dtype=np.float64)
    scale = n_in / n_out
    for i in range(n_out):
        src = (i + 0.5) * scale - 0.5
        f = int(np.floor(src))
        w = src - f
        lo = min(max(f, 0), n_in - 1)
        hi = min(max(f + 1, 0), n_in - 1)
        U[i, lo] += 1.0 - w
        U[i, hi] += w
    return U.astype(np.float32)


def _build_program(stage: int = 99):
    import concourse.bass as bass
    import concourse.bacc as bacc
    import concourse.tile as tile
    from concourse import mybir
    from contextlib import ExitStack

    f16 = mybir.dt.float16
    f32 = mybir.dt.float32
    OP = mybir.AluOpType

    nc = bacc.Bacc("TRN2", target_bir_lowering=False, debug=False,
                   num_devices=N_CORES)

    preds_d = nc.dram_tensor("preds", [K, HF, WF], f32, kind="ExternalInput")
    mask_d = nc.dram_tensor("mask", [2, 128, WF], f16, kind="ExternalInput")
    feats_d = nc.dram_tensor("feats", [PIX, C + 1], f16, kind="ExternalInput")
    u_d = nc.dram_tensor("u", [2, 128, HC], f16, kind="ExternalInput")
    ucat_d = nc.dram_tensor("ucat", [2, 128, 128], f16, kind="ExternalInput")
    ident_d = nc.dram_tensor("ident", [64, 64], f16, kind="ExternalInput")
    out_d = nc.dram_tensor("out", [K, C + 1], f32, kind="ExternalOutput")

    with tile.TileContext(nc) as tc, ExitStack() as ctx:
        const_pool = ctx.enter_context(tc.tile_pool(name="const", bufs=1))
        data_pool = ctx.enter_context(tc.tile_pool(name="data", bufs=1))
        res_pool = ctx.enter_context(tc.tile_pool(name="res", bufs=1))
        ps1_pool = ctx.enter_context(
            tc.tile_pool(name="ps1", bufs=2, space="PSUM"))
        pst_pool = ctx.enter_context(
            tc.tile_pool(name="pst", bufs=1, space="PSUM"))
        psb_pool = ctx.enter_context(
            tc.tile_pool(name="psb", bufs=2, space="PSUM"))
        psf_pool = ctx.enter_context(
            tc.tile_pool(name="psf", bufs=1, space="PSUM"))

        # ---- constants (scalar/ACT DMA ring: tiny, instant) ----
        u16_t = []
        ucat_t = []
        for h in range(2):
            t = const_pool.tile([128, HC], f16, tag=f"u16_{h}")
            nc.scalar.dma_start(t[:], u_d.ap()[h])
            u16_t.append(t)
            t = const_pool.tile([128, 128], f16, tag=f"ucat_{h}")
            nc.scalar.dma_start(t[:], ucat_d.ap()[h])
            ucat_t.append(t)
        ident_t = const_pool.tile([64, 64], f16, tag="ident")
        nc.scalar.dma_start(ident_t[:], ident_d.ap()[:, :])

        # ---- input DMAs (sync ring, FIFO = priority order) ----
        # preds stays f32 (fp16 argmax flips a handful of pixels, and one
        # flip in a ~140-pixel class mean already exceeds the 2e-2 gate).
        # Each half arrives in two k-chunks (KA / K-KA classes) so the DVE
        # max-tree starts ~4us earlier than a whole-half DMA would allow.
        mask_t = data_pool.tile([128, 2 * WF], f16, tag="mask")
        nc.sync.dma_start(mask_t[:], mask_d.ap().transpose([1, 0, 2]))
        preds_t = []
        for h in range(2):
            t = data_pool.tile([128, K * WF], f32, tag=f"preds{h}")
            preds_t.append(t)
        for h in range(2):
            tv = preds_t[h][:].rearrange("p (k w) -> p k w", k=K)
            src = preds_d.ap()[:, h * 128:(h + 1) * 128, :].transpose([1, 0, 2])
            nc.sync.dma_start(tv[:, 0:KA, :], src[:, 0:KA, :])
            nc.sync.dma_start(tv[:, KA:K, :], src[:, KA:K, :])
        ft_big = data_pool.tile([128, 32 * (C + 1)], f16, tag="ftbig")
        nc.sync.dma_start(
            ft_big[:].rearrange("p (x c) -> p x c", x=32),
            feats_d.ap().rearrange("(x p) c -> p x c", p=128))

        # ---- one-hot of mask: oh4[p, k, h2, wf] via tensor_scalar (4x) ----
        oh4 = data_pool.tile([128, K * 2 * WF], f16, tag="oh4")
        ohv = oh4[:].rearrange("p (k x) -> p k x", k=K)
        for k in range(K):
            nc.vector.tensor_scalar(
                ohv[:, k, :], mask_t[:], float(k), None, OP.is_equal)
        ohv4 = oh4[:].rearrange("p (k h w) -> p k h w", k=K, h=2)

        # ---- per-half f32 max over classes: trees of TT maxes; each half
        # in two k-chunks ([0,KA) and [KA,K)) matching the DMA chunks so
        # the tree chases the preds DMA. Temps shared across halves. ----
        t5a = data_pool.tile([128, 5 * WF], f32, tag="t5a")
        v5a = t5a[:].rearrange("p (k w) -> p k w", k=5)
        t5b = data_pool.tile([128, 5 * WF], f32, tag="t5b")
        v5b = t5b[:].rearrange("p (k w) -> p k w", k=5)
        t2 = data_pool.tile([128, 2 * WF], f32, tag="t2")
        v2 = t2[:].rearrange("p (k w) -> p k w", k=2)
        m1 = data_pool.tile([128, WF], f32, tag="m1")
        m2 = data_pool.tile([128, WF], f32, tag="m2")
        mA = data_pool.tile([128, WF], f32, tag="mA")
        maxv_t = []
        pv = []
        for h in range(2):
            pvh = preds_t[h][:].rearrange("p (k w) -> p k w", k=K)
            pv.append(pvh)
            mx = data_pool.tile([128, WF], f32, tag=f"maxv_{h}")
            maxv_t.append(mx)

        def _tree(h):
            # chunk A: classes [0, 11) -> mA
            pvh, mx = pv[h], maxv_t[h]
            dve = nc.vector
            dve.tensor_tensor(v5a, pvh[:, 0:5, :], pvh[:, 5:10, :], op=OP.max)
            dve.tensor_tensor(v2, v5a[:, 0:2, :], v5a[:, 2:4, :], op=OP.max)
            dve.tensor_tensor(m1[:], v2[:, 0, :], v2[:, 1, :], op=OP.max)
            dve.tensor_tensor(m2[:], m1[:], v5a[:, 4, :], op=OP.max)
            dve.tensor_tensor(mA[:], m2[:], pvh[:, 10, :], op=OP.max)
            # chunk B: classes [11, 21) -> merge into maxv
            dve.tensor_tensor(v5b, pvh[:, 11:16, :], pvh[:, 16:21, :],
                              op=OP.max)
            dve.tensor_tensor(v2, v5b[:, 0:2, :], v5b[:, 2:4, :], op=OP.max)
            dve.tensor_tensor(m1[:], v2[:, 0, :], v2[:, 1, :], op=OP.max)
            dve.tensor_tensor(m2[:], m1[:], v5b[:, 4, :], op=OP.max)
            dve.tensor_tensor(mx[:], m2[:], mA[:], op=OP.max)

        # ---- chunked eq/mul (DVE) + stage 1 (PE) + a_t copy (ACT) ----
        eq_t = []
        joint_t = []
        for h in range(2):
            eqh = data_pool.tile([128, KW], f16, tag=f"eq{h}")
            eq_t.append(eqh)
            jh = data_pool.tile([128, KW], f16, tag=f"joint{h}")
            joint_t.append(jh)
        eqv = [eq_t[h][:].rearrange("p (k w) -> p k w", k=K) for h in range(2)]
        jv = [joint_t[h][:].rearrange("p (k w) -> p k w", k=K)
              for h in range(2)]

        a_t = data_pool.tile([64, KW], f16, tag="a")
        at_big = data_pool.tile([128, K * 2 * HC], f16, tag="at")
        atv = at_big[:].rearrange("p (k v h) -> p k v h", k=K, v=2)

        n_kc = (K + 1) // 2  # 11 stage-1 chunks of 2 classes (last has 1)

        # DVE stream: tree of half h, then its eq/mul granules (3 of 7
        # classes — few instructions, per-op overhead is ~160ns), then the
        # next half — h0 compute overlaps the h1 preds DMA.
        NG = 7
        for h in range(2):
            _tree(h)
            for g0 in range(0, K if stage >= 2 else 0, NG):
                bc = (maxv_t[h][:].unsqueeze(1).to_broadcast([128, NG, WF]))
                nc.vector.tensor_tensor(
                    eqv[h][:, g0:g0 + NG, :], pv[h][:, g0:g0 + NG, :],
                    bc, op=OP.is_ge)
                nc.vector.tensor_tensor(
                    jv[h][:, g0:g0 + NG, :], eqv[h][:, g0:g0 + NG, :],
                    ohv4[:, g0:g0 + NG, h, :], op=OP.mult)

        # PE stream: per chunk, accumulate both halves (pairs are emitted
        # together so at most `bufs` PSUM chunks are open at a time), copy
        # out (ACT, casts to fp16), transpose the chunk's classes.
        for kc in range(n_kc if stage >= 2 else 0):
            k0 = 2 * kc
            nk = min(2, K - k0)
            w = nk * WF
            fc = k0 * WF
            ps = ps1_pool.tile([64, 512], f32, tag="ps1")
            nc.tensor.matmul(ps[:, :w], u16_t[0][:, :],
                             joint_t[0][:, fc:fc + w],
                             start=True, stop=False)
            nc.tensor.matmul(ps[:, :w], u16_t[1][:, :],
                             joint_t[1][:, fc:fc + w],
                             start=False, stop=True)
            nc.scalar.copy(a_t[:, fc:fc + w], ps[:, :w])
            # transpose the chunk's classes: at[wf, k, v, hc] = A[hc, k, wf]
            pst = pst_pool.tile([128, 256], f16, tag="pst")
            for u in range(2 * nk):
                k = k0 + u // 2
                wh = u % 2
                nc.tensor.transpose(
                    pst[:, u * 64:(u + 1) * 64],
                    a_t[:, k * WF + wh * 128: k * WF + wh * 128 + 128],
                    ident_t[:])
            nc.scalar.copy(
                atv[:, k0:k0 + nk, :, :],
                pst[:, :nk * 128].rearrange("p (n v h) -> p n v h", n=nk, v=2))

        # ---- stage 2: ds on 128 partitions via [U|U] stationary ----
        # psb[c, (k, hc)] with c = (dup, wc); k-outer/hc-inner keeps the
        # moving operand runs contiguous. b_sh holds (hc, k) with the odd
        # hc rows shifted into the upper partitions (in-partition free-dim
        # shift done by the copies; the hi copy runs on the idle DVE).
        b_sh = data_pool.tile([128, HC * K], f16, tag="bsh")
        bshv = b_sh[:].rearrange("p (h k) -> p h k", h=HC)
        atm = at_big[:].rearrange("p (k v h) -> p v k h", k=K, v=2)
        for c in range(4 if stage >= 4 else 0):
            h0 = 16 * c
            nh = 17 if c < 3 else 16  # one-row overlap feeds the odd shift
            psb = psb_pool.tile([128, K * 17], f32, tag="psb")
            pbv = psb[:].rearrange("p (k h) -> p k h", h=17)
            nc.tensor.matmul(pbv[:, :, :nh], ucat_t[0][:, :],
                             atm[:, 0, :, h0:h0 + nh],
                             start=True, stop=False)
            nc.tensor.matmul(pbv[:, :, :nh], ucat_t[1][:, :],
                             atm[:, 1, :, h0:h0 + nh],
                             start=False, stop=True)
            # transposed-view copies: psb (k, h) -> b_sh (h, k)
            nc.scalar.copy(bshv[0:64, h0:h0 + 16, :],
                           pbv[0:64, :, 0:16].transpose([0, 2, 1]))
            nhi = 16 if c < 3 else 15
            nc.vector.tensor_copy(bshv[64:128, h0:h0 + nhi, :],
                                  pbv[64:128, :, 1:1 + nhi]
                                  .transpose([0, 2, 1]))

        # ---- final: out[k, c] = sum_q ds[q, k] feats^T[q, c]; the 257th
        # feats column is 1.0 so column C lands counts[k] for free ----
        if stage >= 5:
            ftv = ft_big[:].rearrange("p (x c) -> p x c", x=32)
            psf = psf_pool.tile([K, C + 1], f32, tag="fin")
            for ch in range(32):
                nc.tensor.matmul(
                    psf[:, :],
                    bshv[:, 2 * ch, :],
                    ftv[:, ch, :],
                    start=(ch == 0), stop=(ch == 31))
            res_t = res_pool.tile([K, C + 1], f32, tag="res")
            nc.scalar.copy(res_t[:], psf[:])
            nc.sync.dma_start(out_d.ap()[:, :], res_t[:])

    nc.compile()
    return nc


def _get_program():
    if "nc" not in _PROGRAM_CACHE:
        _PROGRAM_CACHE["nc"] = _build_program()
    return _PROGRAM_CACHE["nc"]


def _host_inputs(feats, preds, masks):
    U = _upsample_matrix(HC, HF)  # [256, 64] f32, entries multiples of 1/8
    u16 = U.reshape(2, 128, HC).astype(np.float16)
    ucat = np.concatenate([u16, u16], axis=2)  # [2, 128, 128]
    ident = np.eye(64, dtype=np.float16)

    preds32 = np.asarray(preds, dtype=np.float32)
    mask16 = np.asarray(masks).astype(np.float16).reshape(B, 2, 128, WF)
    feats32 = np.asarray(feats, dtype=np.float32).reshape(B, C, PIX)

    in_maps = []
    for b in range(B):
        ft = np.empty((PIX, C + 1), dtype=np.float16)
        ft[:, :C] = feats32[b].T
        ft[:, C] = 1.0  # ones column -> counts fall out of the final matmul
        in_maps.append({
            "preds": np.ascontiguousarray(preds32[b]),
            "mask": np.ascontiguousarray(mask16[b]),
            "feats": ft,
            "u": u16,
            "ucat": ucat,
            "ident": ident,
        })
    return in_maps


def kernel(feats, preds, masks, _results_hook=None):
    from concourse.bass_utils import run_bass_kernel_spmd

    nc = _get_program()
    in_maps = _host_inputs(feats, preds, masks)
    res = run_bass_kernel_spmd(nc, in_maps, list(range(N_CORES)))
    if _results_hook is not None:
        _results_hook(res)

    protos = []
    for b in range(B):
        out = res.results[b]["out"]  # [K, C+1] f32
        sums = out[:, :C]            # [K, C]
        counts = out[:, C]           # [K]
        protos.append(sums / (counts + EPS)[:, None])  # [K, C]
    return np.mean(np.stack(protos), axis=0).astype(np.float32)
